# revision 3
# baseline (speedup 1.0000x reference)
"""Enformer dot-product self-attention, 8 TRN2 cores, one head per core.

v3 pipeline (from the v2 [j,i]-transposed design):
  - Band rel-shift: T[i,c] tiles -> DRAM G (pitch Q), read back through the
    DMA XBAR transpose with a diagonal [[Q-1,wdt],[1,128]] pattern, landing
    in [j, i] layout.
  - v3 changes vs v2 (118us):
    * G writes moved to the gpsimd SWDGE ring; the sync HWDGE ring carries
      only input loads + the 16 XBAR reads, so write and read wires overlap
      (cross-ring W->R deps come from tile's DRAM range tracking).
    * All 16 XBAR band reads prefetch into a persistent SBUF buffer sb_BT
      right after phase A, so the C sweep never blocks on a mid-loop DMA.
    * attn@v is fused into the C sweep: after exp(jb) lands in sb_PT, four
      512-col matmuls accumulate v^T @ P^T into a persistent PSUM tile po
      [65, 2048] (start at jb==0, stop at jb==15).  Removes the serial
      20us AV phase; softmax denominators ride in PSUM row 64 via the
      ones column of vaug.
    * A-phase PSUM evacuations split column-wise ACT/DVE for engine balance.
  - PSUM budget: psQ 2x[128,1024] (8KB) + po [65,2048] (8KB) = 16KB exact;
    the F-phase transpose pool opens only after psQ closes.
"""

import numpy as np
import ml_dtypes

import concourse.bass as bass
import concourse.bacc as bacc
import concourse.mybir as mybir
import concourse.tile as tile
from concourse.bass_utils import run_bass_kernel_spmd
from concourse.masks import make_identity

S = 2048
D = 64
NB = 64
H = 8
HALF = NB // 2
BAND = 1024
Q = S + 128      # G row pitch (2049 band cols + 127 zero pad)
NT = S // 128
F32 = mybir.dt.float32
F32R = mybir.dt.float32r
BF16 = mybir.dt.bfloat16

_NC_CACHE = {}

# per j-tile XBAR-read geometry: (col offset into sb_BT, ilo, ihi, wdt)
_BT = []
_off = 0
for _jb in range(NT):
    _j0 = _jb * 128
    _ilo = max(0, _j0 - BAND)
    _ihi = min(S, _j0 + 128 + BAND)
    _BT.append((_off, _ilo, _ihi, _ihi - _ilo))
    _off += _ihi - _ilo
BT_COLS = _off  # 25600


def _basis_feature_matrix():
    pow_rate = np.float32(np.exp(np.log((S + 1) / 2) / HALF))
    widths = np.power(pow_rate, np.arange(1, HALF + 1, dtype=np.float32),
                      dtype=np.float32)
    d = (np.float32(BAND) - np.arange(Q, dtype=np.float32))[:, None]
    unsigned = (np.abs(d) <= widths[None, :]).astype(np.float32)
    signed = np.sign(d) * unsigned
    return np.concatenate([unsigned, signed], axis=-1)  # [Q, 64]


def _build_nc():
    if "nc" in _NC_CACHE:
        return _NC_CACHE["nc"]

    nc = bacc.Bacc("TRN2", target_bir_lowering=False, debug=False,
                   num_devices=H)
    d_qf = nc.dram_tensor("qaug_f", [65, S], F32R, kind="ExternalInput")
    d_qb = nc.dram_tensor("qaug_b", [65, S], BF16, kind="ExternalInput")
    d_k = nc.dram_tensor("kaug", [65, S], F32R, kind="ExternalInput")
    d_w2r = nc.dram_tensor("w2r", [65, Q], BF16, kind="ExternalInput")
    d_v = nc.dram_tensor("vaug", [S, 65], BF16, kind="ExternalInput")
    d_out = nc.dram_tensor("out", [S, D], F32, kind="ExternalOutput")
    d_G = nc.dram_tensor("gband", [S * Q], BF16, kind="Internal")

    with tile.TileContext(nc) as tc:
        with tc.tile_pool(name="pers", bufs=1) as pers:
            # sync ring: A-phase inputs first (qb tile 0, then w2r chunks
            # covering A(0)'s band first, then the rest).
            sb_qb = pers.tile([65, S], BF16)
            nc.sync.dma_start(out=sb_qb[:, 0:128], in_=d_qb[:, 0:128])
            sb_w2r = pers.tile([65, Q], BF16)
            for c in (1, 2, 3, 0):
                lo, hi = c * 544, min(Q, (c + 1) * 544)
                nc.sync.dma_start(out=sb_w2r[:, lo:hi], in_=d_w2r[:, lo:hi])
            nc.sync.dma_start(out=sb_qb[:, 128:S], in_=d_qb[:, 128:S])
            # scalar HWDGE ring: sweep-phase inputs (not needed until C).
            sb_qf = pers.tile([65, S], F32R)
            nc.scalar.dma_start(out=sb_qf[:], in_=d_qf[:])
            sb_k = pers.tile([65, S], F32R)
            nc.scalar.dma_start(out=sb_k[:], in_=d_k[:])
            sb_v = pers.tile([128, NT, 65], BF16)
            rdv = bass.AP(tensor=d_v, offset=0,
                          ap=[[65, 128], [128 * 65, NT], [1, 65]])
            nc.scalar.dma_start(out=sb_v[:], in_=rdv)
            sb_id = pers.tile([128, 128], F32)
            sb_PT = pers.tile([128, NT, S], BF16)   # P^T, [j-part, jb, i]
            sb_BT = pers.tile([128, BT_COLS], BF16)  # bias^T bands, [j, i]

            def phase_A(t):
                i0 = t * 128
                jlo = max(0, i0 - BAND)
                jhi = min(S, i0 + 128 + BAND)
                clo = max(0, (jlo - i0 + BAND) - 127)
                chi = min(2049, (jhi - 1) - i0 + BAND + 1)
                gt = gsb.tile([128, Q], BF16)
                nc.gpsimd.memset(gt[:, chi:Q], 0.0)
                cuts = list(range(clo, chi, 1024)) + [chi]
                for ci in range(len(cuts) - 1):
                    lo, hi = cuts[ci], cuts[ci + 1]
                    pg = psQ.tile([128, 1024], F32, tag="pq")
                    nsub = (hi - lo + 511) // 512
                    for si in range(nsub):
                        slo = lo + si * 512
                        shi = min(hi, slo + 512)
                        nc.tensor.matmul(
                            pg[:, slo - lo:shi - lo],
                            lhsT=sb_qb[:, i0:i0 + 128],
                            rhs=sb_w2r[:, slo:shi],
                            start=True, stop=True)
                    # column-split evacuation: ACT takes ~55%, DVE the rest
                    w = hi - lo
                    a = (w * 5 // 9 + 7) & ~7
                    a = min(a, w)
                    if a > 0:
                        nc.scalar.copy(out=gt[:, lo:lo + a], in_=pg[:, 0:a])
                    if a < w:
                        nc.vector.tensor_copy(gt[:, lo + a:hi],
                                              pg[:, a:w])
                wr = bass.AP(tensor=d_G, offset=i0 * Q + clo,
                             ap=[[Q, 128], [1, Q - clo]])
                nc.gpsimd.dma_start(out=wr, in_=gt[:, clo:Q])

            def phase_Bread(jb):
                boff, ilo, ihi, wdt = _BT[jb]
                j0 = jb * 128
                rd = bass.AP(tensor=d_G, offset=ilo * (Q - 1) + j0 + BAND,
                             ap=[[Q - 1, wdt], [1, 128]])
                nc.sync.dma_start(out=sb_BT[:, boff:boff + wdt], in_=rd,
                                  transpose=True)

            def phase_C(jb):
                boff, ilo, ihi, wdt = _BT[jb]
                j0 = jb * 128
                for hf in range(2):
                    h0 = hf * 1024
                    pq = psQ.tile([128, 1024], F32, tag="pq")
                    alo = max(ilo, h0)
                    ahi = min(ihi, h0 + 1024)
                    for c in range(2):
                        nc.tensor.matmul(
                            pq[:, c * 512:(c + 1) * 512],
                            lhsT=sb_k[:, j0:j0 + 128],
                            rhs=sb_qf[:, h0 + c * 512:h0 + (c + 1) * 512],
                            start=True, stop=True)
                    if alo < ahi:
                        nc.vector.tensor_add(
                            pq[:, alo - h0:ahi - h0],
                            pq[:, alo - h0:ahi - h0],
                            sb_BT[:, boff + alo - ilo:boff + ahi - ilo])
                    nc.scalar.activation(
                        out=sb_PT[:, jb, h0:h0 + 1024], in_=pq[:],
                        func=mybir.ActivationFunctionType.Exp)

            def phase_AV(jb):
                for c in range(4):
                    cs = c * 512
                    nc.tensor.matmul(
                        po[0:65, cs:cs + 512],
                        lhsT=sb_v[:, jb, :],
                        rhs=sb_PT[:, jb, cs:cs + 512],
                        start=(jb == 0), stop=(jb == NT - 1))

            with tc.tile_pool(name="psO", bufs=1, space="PSUM") as psO:
                po = psO.tile([65, S], F32)
                with tc.tile_pool(name="gsb", bufs=3) as gsb, \
                     tc.tile_pool(name="psQ", bufs=2, space="PSUM") as psQ:
                    make_identity(nc, sb_id[:])
                    for t in range(NT):
                        phase_A(t)
                    for jb in range(NT):
                        phase_Bread(jb)
                    for jb in range(NT):
                        phase_C(jb)
                        if jb >= 1:
                            phase_AV(jb - 1)
                    phase_AV(NT - 1)

                with tc.tile_pool(name="osb", bufs=2) as osb, \
                     tc.tile_pool(name="fsb", bufs=2) as fsb, \
                     tc.tile_pool(name="psV", bufs=2, space="PSUM") as psV:
                    for c in range(4):
                        cs = c * 512
                        o = osb.tile([65, 512], F32, tag="oT")
                        nc.scalar.copy(out=o[:], in_=po[0:65, cs:cs + 512])
                        ot = fsb.tile([128, 4, D], F32, tag="ot")
                        for s in range(4):
                            pf = psV.tile([128, 65], F32, tag="pf")
                            nc.tensor.transpose(pf[:, 0:65],
                                                o[:, s * 128:(s + 1) * 128],
                                                sb_id[0:65, 0:65])
                            rc = fsb.tile([128, 1], F32, tag="rc")
                            nc.vector.reciprocal(rc[:], pf[:, 64:65])
                            nc.vector.tensor_scalar_mul(ot[:, s, :],
                                                        pf[:, 0:D], rc[:])
                        wr = bass.AP(tensor=d_out, offset=c * 512 * D,
                                     ap=[[D, 128], [128 * D, 4], [1, D]])
                        nc.gpsimd.dma_start(out=wr, in_=ot[:])

    nc.finalize()
    _NC_CACHE["nc"] = nc
    return nc


def _host_prep(query, key, value, u, v, w):
    q = np.asarray(query, np.float32)[0]
    k = np.asarray(key, np.float32)[0]
    val = np.asarray(value, np.float32)[0]
    u = np.asarray(u, np.float32)
    v = np.asarray(v, np.float32)
    w = np.asarray(w, np.float32)
    Rr = _basis_feature_matrix()

    ones_row = np.ones((1, S), np.float32)
    in_maps = []
    for h in range(H):
        qT8 = np.ascontiguousarray(q[:, h, :].T) / np.float32(8.0)
        qaug = np.concatenate([qT8, ones_row], axis=0)
        kT = np.ascontiguousarray(k[:, h, :].T)
        uk8 = ((u[h] / np.float32(8.0)) @ kT).reshape(1, S)
        kaug = np.concatenate([kT, uk8], axis=0)
        vaug = np.concatenate([val[:, h, :], np.ones((S, 1), np.float32)],
                              axis=1).astype(ml_dtypes.bfloat16)
        w2r_qr = w[h] @ Rr.T
        vw8 = (v[h] @ w[h]) / np.float32(8.0)
        w2r_vr = (vw8 @ Rr.T).reshape(1, Q)
        w2r = np.concatenate([w2r_qr, w2r_vr],
                             axis=0).astype(ml_dtypes.bfloat16)
        in_maps.append({
            "qaug_f": np.ascontiguousarray(qaug),
            "qaug_b": np.ascontiguousarray(qaug).astype(ml_dtypes.bfloat16),
            "kaug": np.ascontiguousarray(kaug),
            "vaug": np.ascontiguousarray(vaug),
            "w2r": np.ascontiguousarray(w2r),
        })
    return in_maps


def kernel(query, key, value, u, v, w, _trace=False):
    nc = _build_nc()
    in_maps = _host_prep(query, key, value, u, v, w)
    res = run_bass_kernel_spmd(nc, in_maps, core_ids=list(range(H)),
                               trace=_trace)
    outs = np.stack([res.results[h]["out"] for h in range(H)])
    full = np.transpose(outs, (1, 0, 2))[None]
    out = np.ascontiguousarray(full.astype(np.float32))
    if _trace:
        return out, res
    return out


if __name__ == "__main__":
    rng = np.random.default_rng(0)
    ins = {
        "query": rng.standard_normal((1, S, H, D), np.float32),
        "key": rng.standard_normal((1, S, H, D), np.float32),
        "value": rng.standard_normal((1, S, H, D), np.float32),
        "u": rng.standard_normal((H, D), np.float32),
        "v": rng.standard_normal((H, D), np.float32),
        "w": rng.standard_normal((H, D, NB), np.float32),
    }
    out = kernel(**ins)
    print("out shape:", out.shape, "finite:", np.isfinite(out).all())


# revision 6
# speedup vs baseline: 1.0302x; 1.0302x over previous
"""Enformer dot-product self-attention, 8 TRN2 cores, one head per core.

v3 pipeline (from the v2 [j,i]-transposed design):
  - Band rel-shift: T[i,c] tiles -> DRAM G (pitch Q), read back through the
    DMA XBAR transpose with a diagonal [[Q-1,wdt],[1,128]] pattern, landing
    in [j, i] layout.
  - v3 changes vs v2 (118us):
    * G writes moved to the gpsimd SWDGE ring; the sync HWDGE ring carries
      only input loads + the 16 XBAR reads, so write and read wires overlap
      (cross-ring W->R deps come from tile's DRAM range tracking).
    * All 16 XBAR band reads prefetch into a persistent SBUF buffer sb_BT
      right after phase A, so the C sweep never blocks on a mid-loop DMA.
    * attn@v is fused into the C sweep: after exp(jb) lands in sb_PT, four
      512-col matmuls accumulate v^T @ P^T into a persistent PSUM tile po
      [65, 2048] (start at jb==0, stop at jb==15).  Removes the serial
      20us AV phase; softmax denominators ride in PSUM row 64 via the
      ones column of vaug.
    * A-phase PSUM evacuations split column-wise ACT/DVE for engine balance.
  - PSUM budget: psQ 2x[128,1024] (8KB) + po [65,2048] (8KB) = 16KB exact;
    the F-phase transpose pool opens only after psQ closes.
"""

import numpy as np
import ml_dtypes

import concourse.bass as bass
import concourse.bacc as bacc
import concourse.mybir as mybir
import concourse.tile as tile
from concourse.bass_utils import run_bass_kernel_spmd
from concourse.masks import make_identity

S = 2048
D = 64
NB = 64
H = 8
HALF = NB // 2
BAND = 1024
Q = S + 128      # G row pitch (2049 band cols + 127 zero pad)
NT = S // 128
F32 = mybir.dt.float32
F32R = mybir.dt.float32r
BF16 = mybir.dt.bfloat16

_NC_CACHE = {}

# per j-tile XBAR-read geometry: (col offset into sb_BT, ilo, ihi, wdt)
_BT = []
_off = 0
for _jb in range(NT):
    _j0 = _jb * 128
    _ilo = max(0, _j0 - BAND)
    _ihi = min(S, _j0 + 128 + BAND)
    _BT.append((_off, _ilo, _ihi, _ihi - _ilo))
    _off += _ihi - _ilo
BT_COLS = _off  # 25600


def _basis_feature_matrix():
    pow_rate = np.float32(np.exp(np.log((S + 1) / 2) / HALF))
    widths = np.power(pow_rate, np.arange(1, HALF + 1, dtype=np.float32),
                      dtype=np.float32)
    d = (np.float32(BAND) - np.arange(Q, dtype=np.float32))[:, None]
    unsigned = (np.abs(d) <= widths[None, :]).astype(np.float32)
    signed = np.sign(d) * unsigned
    return np.concatenate([unsigned, signed], axis=-1)  # [Q, 64]


def _build_nc():
    if "nc" in _NC_CACHE:
        return _NC_CACHE["nc"]

    nc = bacc.Bacc("TRN2", target_bir_lowering=False, debug=False,
                   num_devices=H)
    d_qf = nc.dram_tensor("qaug_f", [65, S], F32R, kind="ExternalInput")
    d_qb = nc.dram_tensor("qaug_b", [65, S], BF16, kind="ExternalInput")
    d_k = nc.dram_tensor("kaug", [65, S], F32R, kind="ExternalInput")
    d_w2r = nc.dram_tensor("w2r", [65, Q], BF16, kind="ExternalInput")
    d_v = nc.dram_tensor("vaug", [S, 65], BF16, kind="ExternalInput")
    d_out = nc.dram_tensor("out", [S, D], F32, kind="ExternalOutput")
    d_G = nc.dram_tensor("gband", [S * Q], BF16, kind="Internal")

    with tile.TileContext(nc) as tc:
        with tc.tile_pool(name="pers", bufs=1) as pers:
            # sync ring: A-phase inputs first (qb tile 0, then w2r chunks
            # covering A(0)'s band first, then the rest).
            sb_qb = pers.tile([65, S], BF16)
            nc.sync.dma_start(out=sb_qb[:, 0:128], in_=d_qb[:, 0:128])
            sb_w2r = pers.tile([65, Q], BF16)
            for c in (1, 2, 3, 0):
                lo, hi = c * 544, min(Q, (c + 1) * 544)
                nc.sync.dma_start(out=sb_w2r[:, lo:hi], in_=d_w2r[:, lo:hi])
            nc.sync.dma_start(out=sb_qb[:, 128:S], in_=d_qb[:, 128:S])
            # scalar HWDGE ring: sweep-phase inputs (not needed until C).
            sb_qf = pers.tile([65, S], F32R)
            nc.scalar.dma_start(out=sb_qf[:], in_=d_qf[:])
            sb_k = pers.tile([65, S], F32R)
            nc.scalar.dma_start(out=sb_k[:], in_=d_k[:])
            sb_v = pers.tile([128, NT, 65], BF16)
            rdv = bass.AP(tensor=d_v, offset=0,
                          ap=[[65, 128], [128 * 65, NT], [1, 65]])
            nc.scalar.dma_start(out=sb_v[:], in_=rdv)
            sb_id = pers.tile([128, 128], F32)
            sb_PT = pers.tile([128, NT, S], BF16)   # P^T, [j-part, jb, i]
            sb_BT = pers.tile([128, BT_COLS], BF16)  # bias^T bands, [j, i]

            def phase_A(t):
                i0 = t * 128
                jlo = max(0, i0 - BAND)
                jhi = min(S, i0 + 128 + BAND)
                clo = max(0, (jlo - i0 + BAND) - 127)
                chi = min(2049, (jhi - 1) - i0 + BAND + 1)
                gt = gsb.tile([128, Q], BF16)
                nc.gpsimd.memset(gt[:, chi:Q], 0.0)
                cuts = list(range(clo, chi, 1024)) + [chi]
                for ci in range(len(cuts) - 1):
                    lo, hi = cuts[ci], cuts[ci + 1]
                    pg = psQ.tile([128, 1024], F32, tag="pq")
                    nsub = (hi - lo + 511) // 512
                    for si in range(nsub):
                        slo = lo + si * 512
                        shi = min(hi, slo + 512)
                        nc.tensor.matmul(
                            pg[:, slo - lo:shi - lo],
                            lhsT=sb_qb[:, i0:i0 + 128],
                            rhs=sb_w2r[:, slo:shi],
                            start=True, stop=True)
                    # column-split evacuation: ACT ~40% (it also issues the
                    # G-write DGE), DVE the rest
                    w = hi - lo
                    a = (w * 2 // 5 + 7) & ~7
                    a = min(a, w)
                    if a > 0:
                        nc.scalar.copy(out=gt[:, lo:lo + a], in_=pg[:, 0:a])
                    if a < w:
                        nc.vector.tensor_copy(gt[:, lo + a:hi],
                                              pg[:, a:w])
                wr = bass.AP(tensor=d_G, offset=i0 * Q + clo,
                             ap=[[Q, 128], [1, Q - clo]])
                nc.scalar.dma_start(out=wr, in_=gt[:, clo:Q])

            def phase_Bread(jb):
                boff, ilo, ihi, wdt = _BT[jb]
                j0 = jb * 128
                rd = bass.AP(tensor=d_G, offset=ilo * (Q - 1) + j0 + BAND,
                             ap=[[Q - 1, wdt], [1, 128]])
                nc.sync.dma_start(out=sb_BT[:, boff:boff + wdt], in_=rd,
                                  transpose=True)

            def phase_C(jb):
                boff, ilo, ihi, wdt = _BT[jb]
                j0 = jb * 128
                for hf in range(2):
                    h0 = hf * 1024
                    pq = psQ.tile([128, 1024], F32, tag="pq")
                    alo = max(ilo, h0)
                    ahi = min(ihi, h0 + 1024)
                    for c in range(2):
                        nc.tensor.matmul(
                            pq[:, c * 512:(c + 1) * 512],
                            lhsT=sb_k[:, j0:j0 + 128],
                            rhs=sb_qf[:, h0 + c * 512:h0 + (c + 1) * 512],
                            start=True, stop=True)
                    if alo < ahi:
                        nc.vector.tensor_add(
                            pq[:, alo - h0:ahi - h0],
                            pq[:, alo - h0:ahi - h0],
                            sb_BT[:, boff + alo - ilo:boff + ahi - ilo])
                    nc.scalar.activation(
                        out=sb_PT[:, jb, h0:h0 + 1024], in_=pq[:],
                        func=mybir.ActivationFunctionType.Exp)

            def phase_AV(jb):
                for c in range(4):
                    cs = c * 512
                    nc.tensor.matmul(
                        po[0:65, cs:cs + 512],
                        lhsT=sb_v[:, jb, :],
                        rhs=sb_PT[:, jb, cs:cs + 512],
                        start=(jb == 0), stop=(jb == NT - 1))

            with tc.tile_pool(name="psO", bufs=1, space="PSUM") as psO:
                po = psO.tile([65, S], F32)
                with tc.tile_pool(name="gsb", bufs=3) as gsb, \
                     tc.tile_pool(name="psQ", bufs=2, space="PSUM") as psQ:
                    make_identity(nc, sb_id[:])
                    # interleave XBAR reads: read jb fires right after the
                    # last write tile its row range depends on
                    for t in range(NT):
                        phase_A(t)
                        if t >= 8:
                            phase_Bread(t - 8)
                    for jb in range(NT - 8, NT):
                        phase_Bread(jb)
                    for jb in range(NT):
                        phase_C(jb)
                        if jb >= 1:
                            phase_AV(jb - 1)
                    phase_AV(NT - 1)

                with tc.tile_pool(name="osb", bufs=2) as osb, \
                     tc.tile_pool(name="fsb", bufs=2) as fsb, \
                     tc.tile_pool(name="psV", bufs=2, space="PSUM") as psV:
                    for c in range(4):
                        cs = c * 512
                        o = osb.tile([65, 512], F32, tag="oT")
                        nc.scalar.copy(out=o[:], in_=po[0:65, cs:cs + 512])
                        ot = fsb.tile([128, 4, D], F32, tag="ot")
                        for s in range(4):
                            pf = psV.tile([128, 65], F32, tag="pf")
                            nc.tensor.transpose(pf[:, 0:65],
                                                o[:, s * 128:(s + 1) * 128],
                                                sb_id[0:65, 0:65])
                            rc = fsb.tile([128, 1], F32, tag="rc")
                            nc.vector.reciprocal(rc[:], pf[:, 64:65])
                            nc.vector.tensor_scalar_mul(ot[:, s, :],
                                                        pf[:, 0:D], rc[:])
                        wr = bass.AP(tensor=d_out, offset=c * 512 * D,
                                     ap=[[D, 128], [128 * D, 4], [1, D]])
                        nc.sync.dma_start(out=wr, in_=ot[:])

    nc.finalize()
    _NC_CACHE["nc"] = nc
    return nc


def _host_prep(query, key, value, u, v, w):
    q = np.asarray(query, np.float32)[0]
    k = np.asarray(key, np.float32)[0]
    val = np.asarray(value, np.float32)[0]
    u = np.asarray(u, np.float32)
    v = np.asarray(v, np.float32)
    w = np.asarray(w, np.float32)
    Rr = _basis_feature_matrix()

    ones_row = np.ones((1, S), np.float32)
    in_maps = []
    for h in range(H):
        qT8 = np.ascontiguousarray(q[:, h, :].T) / np.float32(8.0)
        qaug = np.concatenate([qT8, ones_row], axis=0)
        kT = np.ascontiguousarray(k[:, h, :].T)
        uk8 = ((u[h] / np.float32(8.0)) @ kT).reshape(1, S)
        kaug = np.concatenate([kT, uk8], axis=0)
        vaug = np.concatenate([val[:, h, :], np.ones((S, 1), np.float32)],
                              axis=1).astype(ml_dtypes.bfloat16)
        w2r_qr = w[h] @ Rr.T
        vw8 = (v[h] @ w[h]) / np.float32(8.0)
        w2r_vr = (vw8 @ Rr.T).reshape(1, Q)
        w2r = np.concatenate([w2r_qr, w2r_vr],
                             axis=0).astype(ml_dtypes.bfloat16)
        in_maps.append({
            "qaug_f": np.ascontiguousarray(qaug),
            "qaug_b": np.ascontiguousarray(qaug).astype(ml_dtypes.bfloat16),
            "kaug": np.ascontiguousarray(kaug),
            "vaug": np.ascontiguousarray(vaug),
            "w2r": np.ascontiguousarray(w2r),
        })
    return in_maps


def kernel(query, key, value, u, v, w, _trace=False):
    nc = _build_nc()
    in_maps = _host_prep(query, key, value, u, v, w)
    res = run_bass_kernel_spmd(nc, in_maps, core_ids=list(range(H)),
                               trace=_trace)
    outs = np.stack([res.results[h]["out"] for h in range(H)])
    full = np.transpose(outs, (1, 0, 2))[None]
    out = np.ascontiguousarray(full.astype(np.float32))
    if _trace:
        return out, res
    return out


if __name__ == "__main__":
    rng = np.random.default_rng(0)
    ins = {
        "query": rng.standard_normal((1, S, H, D), np.float32),
        "key": rng.standard_normal((1, S, H, D), np.float32),
        "value": rng.standard_normal((1, S, H, D), np.float32),
        "u": rng.standard_normal((H, D), np.float32),
        "v": rng.standard_normal((H, D), np.float32),
        "w": rng.standard_normal((H, D, NB), np.float32),
    }
    out = kernel(**ins)
    print("out shape:", out.shape, "finite:", np.isfinite(out).all())
